# revision 1
# baseline (speedup 1.0000x reference)
"""CLIP attention (B=2, S=2048, H=768, 12 heads) on 8 trn2 NeuronCores.

Sharding: data-parallel over batch (2) x tensor-parallel over head groups
(4 groups of 3 heads).  Each core computes, for its (batch, head-group):
    q = x @ Wq_g * 1/sqrt(64) (+ bq_g scaled)      [2048, 192]
    k = x @ Wk_g                                    [2048, 192]
    v = x @ Wv_g                                    [2048, 192]
    per head: P' = exp(q k^T)   (no max subtraction; logits are O(1))
              O'^T, s via ones-augmented V:  o_ps = [V_h | 1]^T @ P'^T
    y_partial = sum_h (O'_h / s_h) @ Wo_h           [2048, 768]
      (1/s_h applied on the per-head Y psum, where the query index is the
       partition dim, via tensor_scalar with a per-partition scalar)
Host sums the 4 head-group partials per batch and adds the exactly-folded
bias terms (bk drops out of softmax; bv/bo fold to a constant row).

Matmul convention: nc.tensor.matmul(out, lhsT, rhs) => out = lhsT.T @ rhs,
contraction over the partition dim of both operands.

Scheduling/layout tricks:
  - x^T via DMA xbar transpose (bf16), split across both HWDGE queues.
  - q^T/k^T stored twice (partitions 0-63 / 64-127) so two K=64 logits
    matmuls run concurrently in disjoint PE row groups.
  - attention runs in (head, query-half) passes so its PSUM footprint is
    6 banks (2x lt + o_ps), leaving a 2-bank "flex" slot open; the head-2
    q/k projections and the per-head Y stages are drip-fed into the
    ACT-bound attention loop's PE slack through that slot.
  - s (denominators) transposed per head to [128, 16] via a DRAM bounce,
    then reciprocal_approx_fast -> per-partition scalars for Y.
"""

import sys

if "/opt/trn_rl_repo" not in sys.path:
    sys.path.insert(0, "/opt/trn_rl_repo")

from collections import deque

import numpy as np
import ml_dtypes

import concourse.bacc as bacc
import concourse.tile as tile
from concourse import mybir
from concourse.bass_utils import run_bass_kernel_spmd

BF16 = mybir.dt.bfloat16
F32 = mybir.dt.float32

S = 2048          # sequence length
C = 768           # hidden
NH = 12           # total heads
HD = 64           # head dim
NCORES = 8
GROUPS = 4        # head groups (tensor parallel)
HPG = NH // GROUPS          # heads per group = 3
GF = HPG * HD               # group feature width = 192
NCC = C // 128              # contraction chunks = 6
NQB = S // 128              # token blocks = 16
NKB = S // 128              # key blocks = 16


def build_program():
    nc = bacc.Bacc("TRN2", target_bir_lowering=False, debug=False)

    x = nc.dram_tensor("x", (S, C), BF16, kind="ExternalInput").ap()
    wq = nc.dram_tensor("wq", (C, GF), BF16, kind="ExternalInput").ap()
    wk = nc.dram_tensor("wk", (C, GF), BF16, kind="ExternalInput").ap()
    wv = nc.dram_tensor("wv", (C, GF), BF16, kind="ExternalInput").ap()
    wo = nc.dram_tensor("wo", (GF, C), BF16, kind="ExternalInput").ap()
    bq = nc.dram_tensor("bq", (1, GF), BF16, kind="ExternalInput").ap()
    out = nc.dram_tensor("out", (S, C), F32, kind="ExternalOutput").ap()

    with tile.TileContext(nc) as tc:
        with tc.tile_pool(name="consts", bufs=1) as consts, \
             tc.tile_pool(name="dram", bufs=1, space="DRAM") as dram_pool:
            # weights on the Scalar HWDGE queue, x^T transposes split over
            # both HWDGE queues so everything loads in parallel
            wq_sb = consts.tile([128, NCC, GF], BF16)
            wk_sb = consts.tile([128, NCC, GF], BF16)
            wv_sb = consts.tile([128, NCC, GF], BF16)
            for w_sb, w_dram in ((wq_sb, wq), (wk_sb, wk), (wv_sb, wv)):
                for c in range(NCC):
                    nc.scalar.dma_start(
                        out=w_sb[:, c, :], in_=w_dram[c * 128 : (c + 1) * 128, :]
                    )
            wo_t = [consts.tile([HD, C], BF16, name=f"wo{h}") for h in range(HPG)]
            for h in range(HPG):
                nc.scalar.dma_start(
                    out=wo_t[h][:], in_=wo[h * HD : (h + 1) * HD, :]
                )
            bq_sb = consts.tile([1, GF], BF16)
            nc.scalar.dma_start(out=bq_sb[:], in_=bq[:])
            xT = [consts.tile([128, S], BF16, name=f"xT{c}") for c in range(NCC)]
            for c in range(NCC):
                eng = nc.sync if c % 2 == 0 else nc.scalar
                eng.dma_start_transpose(
                    out=xT[c][:], in_=x[:, c * 128 : (c + 1) * 128]
                )

            ones_bf = consts.tile([1, 512], BF16)
            nc.vector.memset(ones_bf[:], 1.0)

            qT = [consts.tile([128, S], BF16, name=f"qT{h}") for h in range(HPG)]
            kT = [consts.tile([128, S], BF16, name=f"kT{h}") for h in range(HPG)]
            vS = [consts.tile([128, HPG, HD + 1], BF16, name=f"vS{t}")
                  for t in range(NKB)]
            # oT per head, split in query halves so drip-fed consumers can
            # start after the first half is written
            oT = [[consts.tile([HD, S // 2], BF16, name=f"oT{h}_{qh}")
                   for qh in range(2)] for h in range(HPG)]
            ys2_sb = [consts.tile([128, C], F32, name=f"ys2_{t}")
                      for t in range(NQB // 2)]
            s_row = [consts.tile([1, S], F32, name=f"s_row{h}") for h in range(HPG)]
            s_dram = dram_pool.tile([HPG, S], F32)
            sT = [consts.tile([128, NQB], F32, name=f"sT{h}") for h in range(HPG)]
            rT = [consts.tile([128, NQB], F32, name=f"rT{h}") for h in range(HPG)]
            ys = [consts.tile([128, C], F32, name=f"ys{t}") for t in range(NQB)]

            def qk_proj_piece(w_sb, dst_list, with_bias, pool, tag, m0, mw, n,
                              heads):
                """One [mw, 512] projection piece -> per-head q^T/k^T rows."""
                ps = pool.tile([mw, 512], F32, tag=tag, name=f"qkp_{m0}_{n}")
                for c in range(NCC):
                    nc.tensor.matmul(
                        ps[:],
                        w_sb[:, c, m0 : m0 + mw],
                        xT[c][:, n * 512 : (n + 1) * 512],
                        start=(c == 0),
                        stop=(c == NCC - 1 and not with_bias),
                    )
                if with_bias:
                    nc.tensor.matmul(
                        ps[:], bq_sb[:, m0 : m0 + mw], ones_bf[:],
                        start=False, stop=True,
                    )
                n0 = n * 512
                for i, h in enumerate(heads):
                    nc.vector.tensor_copy(
                        dst_list[h][0:64, n0 : n0 + 512],
                        ps[i * 64 : (i + 1) * 64, :],
                    )

            def v_piece(pool, tag, t):
                vps = pool.tile([128, GF], F32, tag=tag, name=f"vp{t}")
                for c in range(NCC):
                    nc.tensor.matmul(
                        vps[:],
                        xT[c][:, t * 128 : (t + 1) * 128],
                        wv_sb[:, c, :],
                        start=(c == 0),
                        stop=(c == NCC - 1),
                    )
                nc.vector.tensor_copy(
                    vS[t][:, :, 0:HD],
                    vps[:].rearrange("p (h d) -> p h d", h=HPG),
                )
                nc.vector.memset(vS[t][:, :, HD : HD + 1], 1.0)

            # ---------------- QKV projections (heads 0,1) + V ----------------
            with tc.tile_pool(name="pp", bufs=3, space="PSUM") as pp, \
                 tc.tile_pool(name="vpp", bufs=2, space="PSUM") as vpp:
                for w_sb, dst, with_bias in ((wq_sb, qT, True), (wk_sb, kT, False)):
                    for n in range(S // 512):
                        qk_proj_piece(w_sb, dst, with_bias, pp, "pp_qk",
                                      0, 128, n, (0, 1))
                for h in (0, 1):
                    nc.sync.dma_start(out=qT[h][64:128, :], in_=qT[h][0:64, :])
                    nc.sync.dma_start(out=kT[h][64:128, :], in_=kT[h][0:64, :])
                for t in range(4):
                    v_piece(vpp, "vps", t)

            # ---------- attention + drip-fed projections/Y stages ----------
            with tc.tile_pool(name="flex", bufs=1, space="PSUM") as flex, \
                 tc.tile_pool(name="lt_ps", bufs=2, space="PSUM") as ltp, \
                 tc.tile_pool(name="o_ps", bufs=1, space="PSUM") as opp, \
                 tc.tile_pool(name="att_sb", bufs=4) as asb:
                # background tasks drip-fed into the attention loop's PE slack
                bg = deque()

                def bg_v_piece(t):
                    def run():
                        v_piece(flex, "flex", t)
                    return run

                def bg_m1_piece(w_sb, dst, with_bias, n):
                    def run():
                        qk_proj_piece(w_sb, dst, with_bias, flex, "flex",
                                      128, 64, n, (2,))
                    return run

                def bg_dup_h2():
                    def run():
                        nc.sync.dma_start(out=qT[2][64:128, :], in_=qT[2][0:64, :])
                        nc.sync.dma_start(out=kT[2][64:128, :], in_=kT[2][0:64, :])
                    return run

                def y_mms(h, t, yp):
                    tbs = slice((t % (NQB // 2)) * 128, (t % (NQB // 2) + 1) * 128)
                    for n0, nw in ((0, 512), (512, 256)):
                        nc.tensor.matmul(
                            yp[:, n0 : n0 + nw],
                            oT[h][t // (NQB // 2)][:, tbs],
                            wo_t[h][:, n0 : n0 + nw],
                            start=True,
                            stop=True,
                        )

                def bg_y_step(h, t):
                    def run():
                        yp = flex.tile([128, C], F32, tag="flex",
                                       name=f"yp{h}_{t}")
                        y_mms(h, t, yp)
                        if h == 0:
                            nc.vector.tensor_scalar_mul(
                                ys[t][:], yp[:], rT[0][:, t : t + 1]
                            )
                        else:
                            nc.vector.scalar_tensor_tensor(
                                out=ys[t][:],
                                in0=yp[:],
                                scalar=rT[h][:, t : t + 1],
                                in1=ys[t][:],
                                op0=mybir.AluOpType.mult,
                                op1=mybir.AluOpType.add,
                            )
                    return run

                def bg_y2_begin(t):
                    # head-2 Y matmul staged to SBUF; the 1/s scale + add
                    # happens in the (short) tail once rT[2] exists
                    def run():
                        yp = flex.tile([128, C], F32, tag="flex",
                                       name=f"yp2b_{t}")
                        y_mms(2, t, yp)
                        nc.vector.tensor_copy(ys2_sb[t][:], yp[:])
                    return run

                for t in range(4, NQB):
                    bg.append(bg_v_piece(t))
                for n in range(S // 512):
                    bg.append(bg_m1_piece(wq_sb, qT, True, n))
                for n in range(S // 512):
                    bg.append(bg_m1_piece(wk_sb, kT, False, n))
                bg.append(bg_dup_h2())

                for h in range(HPG):
                    for qh in range(2):
                        if h == 2 and qh == 1:
                            for t in range(NQB // 2):
                                bg.append(bg_y2_begin(t))
                        q0 = qh * 1024
                        o_ps = opp.tile([HD + 1, 1024], F32, tag="o",
                                        name=f"o_ps{h}_{qh}")
                        for kb in range(NKB):
                            kbs = slice(kb * 128, (kb + 1) * 128)
                            lt = ltp.tile([128, 1024], F32, tag="lt")
                            # two concurrent K=64 matmuls in disjoint PE
                            # row groups (rows 0-63 / 64-127)
                            nc.tensor.matmul(
                                lt[:, 0:512],
                                kT[h][0:64, kbs],
                                qT[h][0:64, q0 : q0 + 512],
                                start=True,
                                stop=True,
                            )
                            nc.tensor.matmul(
                                lt[:, 512:1024],
                                kT[h][64:128, kbs],
                                qT[h][64:128, q0 + 512 : q0 + 1024],
                                start=True,
                                stop=True,
                            )
                            elt = asb.tile([128, 1024], BF16, tag="elt")
                            nc.scalar.activation(
                                elt[:], lt[:], mybir.ActivationFunctionType.Exp
                            )
                            for nn in range(2):
                                nc.tensor.matmul(
                                    o_ps[:, nn * 512 : (nn + 1) * 512],
                                    vS[kb][:, h, :],
                                    elt[:, nn * 512 : (nn + 1) * 512],
                                    start=(kb == 0),
                                    stop=(kb == NKB - 1),
                                )
                            if bg:
                                bg.popleft()()
                        nc.vector.tensor_copy(
                            oT[h][qh][:], o_ps[0:HD, :]
                        )
                        nc.vector.tensor_copy(
                            s_row[h][:, q0 : q0 + 1024], o_ps[HD : HD + 1, :]
                        )
                    # transpose s_h to [128, 16] via DRAM bounce + 1/s
                    nc.sync.dma_start(out=s_dram[h : h + 1, :], in_=s_row[h][:])
                    nc.sync.dma_start(
                        out=sT[h][:],
                        in_=s_dram[h : h + 1, :].rearrange(
                            "a (b p) -> p (a b)", p=128
                        ),
                    )
                    nc.vector.reciprocal_approx_fast(out=rT[h][:], in_=sT[h][:])
                    if h < 2:
                        for t in range(NQB):
                            bg.append(bg_y_step(h, t))

                while bg:
                    bg.popleft()()
                # tail: finish head-2 normalization + output
                for t in range(NQB // 2):
                    nc.vector.scalar_tensor_tensor(
                        out=ys[t][:],
                        in0=ys2_sb[t][:],
                        scalar=rT[2][:, t : t + 1],
                        in1=ys[t][:],
                        op0=mybir.AluOpType.mult,
                        op1=mybir.AluOpType.add,
                    )
                    nc.sync.dma_start(
                        out=out[t * 128 : (t + 1) * 128, :], in_=ys[t][:]
                    )
                for t in range(NQB // 2, NQB):
                    yp = flex.tile([128, C], F32, tag="flex", name=f"yp2c_{t}")
                    y_mms(2, t, yp)
                    nc.vector.scalar_tensor_tensor(
                        out=ys[t][:],
                        in0=yp[:],
                        scalar=rT[2][:, t : t + 1],
                        in1=ys[t][:],
                        op0=mybir.AluOpType.mult,
                        op1=mybir.AluOpType.add,
                    )
                    nc.sync.dma_start(
                        out=out[t * 128 : (t + 1) * 128, :], in_=ys[t][:]
                    )

    nc.compile()
    return nc


_COMPILED_NC = None


def _get_nc():
    global _COMPILED_NC
    if _COMPILED_NC is None:
        _COMPILED_NC = build_program()
    return _COMPILED_NC


def make_in_maps(x, Wq, bq, Wk, bk, Wv, bv, Wo, bo):
    scale = 1.0 / np.sqrt(HD)
    bf = ml_dtypes.bfloat16
    x_bf = [np.ascontiguousarray(x[b]).astype(bf) for b in range(x.shape[0])]
    in_maps = []
    for c in range(NCORES):
        b, g = divmod(c, GROUPS)
        cols = slice(g * GF, (g + 1) * GF)
        in_maps.append(
            {
                "x": x_bf[b],
                "wq": np.ascontiguousarray(Wq[:, cols] * scale).astype(bf),
                "wk": np.ascontiguousarray(Wk[:, cols]).astype(bf),
                "wv": np.ascontiguousarray(Wv[:, cols]).astype(bf),
                "wo": np.ascontiguousarray(Wo[cols, :]).astype(bf),
                "bq": np.ascontiguousarray(bq[cols] * scale).reshape(1, GF).astype(bf),
            }
        )
    return in_maps


def gather_output(results, x, Wv, bv, Wo, bo):
    B = x.shape[0]
    out = np.zeros((B, S, C), dtype=np.float32)
    for c in range(NCORES):
        b, _ = divmod(c, GROUPS)
        out[b] += results[c]["out"]
    # exact bias folds: bk cancels in softmax; v-bias -> bv @ Wo; + bo
    out += (np.asarray(bv, np.float32) @ np.asarray(Wo, np.float32)
            + np.asarray(bo, np.float32))
    return out


def kernel(x, Wq, bq, Wk, bk, Wv, bv, Wo, bo):
    x = np.asarray(x)
    nc = _get_nc()
    in_maps = make_in_maps(x, Wq, bq, Wk, bk, Wv, bv, Wo, bo)
    res = run_bass_kernel_spmd(nc, in_maps, core_ids=list(range(NCORES)))
    return gather_output(res.results, x, Wv, bv, Wo, bo)



# revision 5
# speedup vs baseline: 1.1426x; 1.1426x over previous
"""CLIP attention (B=2, S=2048, H=768, 12 heads) on 8 trn2 NeuronCores.

Sharding: data-parallel over batch (2) x tensor-parallel over head groups
(4 groups of 3 heads).  Each core computes, for its (batch, head-group):
    q = x @ Wq_g * 1/sqrt(64) (+ bq_g scaled)      [2048, 192]
    k = x @ Wk_g                                    [2048, 192]
    v = x @ Wv_g                                    [2048, 192]
    per head: P' = exp(q k^T)   (no max subtraction; logits are O(1))
              O'^T, s via ones-augmented V:  o_ps = [V_h | 1]^T @ P'^T
    y_partial = sum_h (O'_h / s_h) @ Wo_h           [2048, 768]
      (1/s_h applied on the per-head Y psum, where the query index is the
       partition dim, via tensor_scalar with a per-partition scalar)
Host sums the 4 head-group partials per batch and adds the exactly-folded
bias terms (bk drops out of softmax; bv/bo fold to a constant row).

Matmul convention: nc.tensor.matmul(out, lhsT, rhs) => out = lhsT.T @ rhs,
contraction over the partition dim of both operands.

Scheduling/layout tricks:
  - x^T via DMA xbar transpose (bf16), split across both HWDGE queues.
  - q^T/k^T stored twice (partitions 0-63 / 64-127) so two K=64 logits
    matmuls run concurrently in disjoint PE row groups.
  - attention runs in (head, query-half) passes so its PSUM footprint is
    6 banks (2x lt + o_ps), leaving a 2-bank "flex" slot open; the head-2
    q/k projections and the per-head Y stages are drip-fed into the
    ACT-bound attention loop's PE slack through that slot.
  - s (denominators) transposed per head to [128, 16] via a DRAM bounce,
    then reciprocal_approx_fast -> per-partition scalars for Y.
"""

import sys

if "/opt/trn_rl_repo" not in sys.path:
    sys.path.insert(0, "/opt/trn_rl_repo")

from collections import deque

import numpy as np
import ml_dtypes

import concourse.bacc as bacc
import concourse.tile as tile
from concourse import mybir
from concourse.bass_utils import run_bass_kernel_spmd

BF16 = mybir.dt.bfloat16
F32 = mybir.dt.float32

S = 2048          # sequence length
C = 768           # hidden
NH = 12           # total heads
HD = 64           # head dim
NCORES = 8
GROUPS = 4        # head groups (tensor parallel)
HPG = NH // GROUPS          # heads per group = 3
GF = HPG * HD               # group feature width = 192
NCC = C // 128              # contraction chunks = 6
NQB = S // 128              # token blocks = 16
NKB = S // 128              # key blocks = 16


def build_program():
    nc = bacc.Bacc("TRN2", target_bir_lowering=False, debug=False)

    x = nc.dram_tensor("x", (S, C), BF16, kind="ExternalInput").ap()
    wq = nc.dram_tensor("wq", (C, GF), BF16, kind="ExternalInput").ap()
    wk = nc.dram_tensor("wk", (C, GF), BF16, kind="ExternalInput").ap()
    wv = nc.dram_tensor("wv", (C, GF), BF16, kind="ExternalInput").ap()
    wo = nc.dram_tensor("wo", (GF, C), BF16, kind="ExternalInput").ap()
    bq = nc.dram_tensor("bq", (1, GF), BF16, kind="ExternalInput").ap()
    out = nc.dram_tensor("out", (S, C), F32, kind="ExternalOutput").ap()

    with tile.TileContext(nc) as tc:
        with tc.tile_pool(name="consts", bufs=1) as consts, \
             tc.tile_pool(name="dram", bufs=1, space="DRAM") as dram_pool:
            # x^T transposes issue FIRST on both HWDGE queues (they gate the
            # first q matmuls); weights follow, interleaved across queues in
            # consumption order (wq, wk, then wv/wo).
            wq_sb = consts.tile([128, NCC, GF], BF16)
            wk_sb = consts.tile([128, NCC, GF], BF16)
            wv_sb = consts.tile([128, NCC, GF], BF16)
            wo_t = [consts.tile([HD, C], BF16, name=f"wo{h}") for h in range(HPG)]
            bq_sb = consts.tile([1, GF], BF16)
            xT = [consts.tile([128, S], BF16, name=f"xT{c}") for c in range(NCC)]
            # All transposes on the sync queue (the XBAR appears to be a
            # shared resource: concurrent transposes on both queues corrupt
            # data); weights load concurrently on the scalar queue.
            for c in range(NCC):
                nc.sync.dma_start_transpose(
                    out=xT[c][:], in_=x[:, c * 128 : (c + 1) * 128]
                )
            nc.scalar.dma_start(out=bq_sb[:], in_=bq[:])
            for w_sb, w_dram in ((wq_sb, wq), (wk_sb, wk), (wv_sb, wv)):
                for c in range(NCC):
                    nc.scalar.dma_start(
                        out=w_sb[:, c, :], in_=w_dram[c * 128 : (c + 1) * 128, :]
                    )
            for h in range(HPG):
                nc.scalar.dma_start(
                    out=wo_t[h][:], in_=wo[h * HD : (h + 1) * HD, :]
                )

            ones_bf = consts.tile([1, 512], BF16)
            nc.vector.memset(ones_bf[:], 1.0)

            qT = [consts.tile([128, S], BF16, name=f"qT{h}") for h in range(HPG)]
            kT = [consts.tile([128, S], BF16, name=f"kT{h}") for h in range(HPG)]
            vS = [consts.tile([128, HPG, HD + 1], BF16, name=f"vS{t}")
                  for t in range(NKB)]
            # oT per head, split in query halves so drip-fed consumers can
            # start after the first half is written
            oT = [[consts.tile([HD, S // 2], BF16, name=f"oT{h}_{qh}")
                   for qh in range(2)] for h in range(HPG)]
            ys2_sb = [consts.tile([128, C], F32, name=f"ys2_{t}")
                      for t in range(NQB // 2)]
            s_row = [consts.tile([1, S], F32, name=f"s_row{h}") for h in range(HPG)]
            s_dram = dram_pool.tile([HPG, S], F32)
            sT = [consts.tile([128, NQB], F32, name=f"sT{h}") for h in range(HPG)]
            rT = [consts.tile([128, NQB], F32, name=f"rT{h}") for h in range(HPG)]
            ys = [consts.tile([128, C], F32, name=f"ys{t}") for t in range(NQB)]

            def qk_proj_piece(w_sb, dst_list, with_bias, pool, tag, m0, mw, n,
                              heads):
                """One [mw, 512] projection piece -> per-head q^T/k^T rows."""
                ps = pool.tile([mw, 512], F32, tag=tag, name=f"qkp_{m0}_{n}")
                for c in range(NCC):
                    nc.tensor.matmul(
                        ps[:],
                        w_sb[:, c, m0 : m0 + mw],
                        xT[c][:, n * 512 : (n + 1) * 512],
                        start=(c == 0),
                        stop=(c == NCC - 1 and not with_bias),
                    )
                if with_bias:
                    nc.tensor.matmul(
                        ps[:], bq_sb[:, m0 : m0 + mw], ones_bf[:],
                        start=False, stop=True,
                    )
                n0 = n * 512
                for i, h in enumerate(heads):
                    nc.vector.tensor_copy(
                        dst_list[h][0:64, n0 : n0 + 512],
                        ps[i * 64 : (i + 1) * 64, :],
                    )

            def v_piece(pool, tag, t):
                vps = pool.tile([128, GF], F32, tag=tag, name=f"vp{t}")
                for c in range(NCC):
                    nc.tensor.matmul(
                        vps[:],
                        xT[c][:, t * 128 : (t + 1) * 128],
                        wv_sb[:, c, :],
                        start=(c == 0),
                        stop=(c == NCC - 1),
                    )
                nc.vector.tensor_copy(
                    vS[t][:, :, 0:HD],
                    vps[:].rearrange("p (h d) -> p h d", h=HPG),
                )
                nc.vector.memset(vS[t][:, :, HD : HD + 1], 1.0)

            # ---------------- QKV projections (heads 0,1) + V ----------------
            with tc.tile_pool(name="pp", bufs=3, space="PSUM") as pp, \
                 tc.tile_pool(name="vpp", bufs=2, space="PSUM") as vpp:
                for w_sb, dst, with_bias in ((wq_sb, qT, True), (wk_sb, kT, False)):
                    for n in range(S // 512):
                        qk_proj_piece(w_sb, dst, with_bias, pp, "pp_qk",
                                      0, 128, n, (0, 1))
                for h in (0, 1):
                    nc.sync.dma_start(out=qT[h][64:128, :], in_=qT[h][0:64, :])
                    nc.sync.dma_start(out=kT[h][64:128, :], in_=kT[h][0:64, :])
                for t in range(4):
                    v_piece(vpp, "vps", t)

            # ---------- attention + drip-fed projections/Y stages ----------
            with tc.tile_pool(name="flex", bufs=1, space="PSUM") as flex, \
                 tc.tile_pool(name="lt_ps", bufs=2, space="PSUM") as ltp, \
                 tc.tile_pool(name="o_ps", bufs=1, space="PSUM") as opp, \
                 tc.tile_pool(name="att_sb", bufs=4) as asb:
                # background tasks drip-fed into the attention loop's PE slack
                bg = deque()

                def bg_v_piece(t):
                    def run():
                        v_piece(flex, "flex", t)
                    return run

                def bg_m1_piece(w_sb, dst, with_bias, n):
                    def run():
                        qk_proj_piece(w_sb, dst, with_bias, flex, "flex",
                                      128, 64, n, (2,))
                    return run

                def bg_dup_h2():
                    def run():
                        nc.sync.dma_start(out=qT[2][64:128, :], in_=qT[2][0:64, :])
                        nc.sync.dma_start(out=kT[2][64:128, :], in_=kT[2][0:64, :])
                    return run

                def y_mms(h, t, yp):
                    tbs = slice((t % (NQB // 2)) * 128, (t % (NQB // 2) + 1) * 128)
                    for n0, nw in ((0, 512), (512, 256)):
                        nc.tensor.matmul(
                            yp[:, n0 : n0 + nw],
                            oT[h][t // (NQB // 2)][:, tbs],
                            wo_t[h][:, n0 : n0 + nw],
                            start=True,
                            stop=True,
                        )

                def bg_y_step(h, t):
                    def run():
                        yp = flex.tile([128, C], F32, tag="flex",
                                       name=f"yp{h}_{t}")
                        y_mms(h, t, yp)
                        if h == 0:
                            nc.vector.tensor_scalar_mul(
                                ys[t][:], yp[:], rT[0][:, t : t + 1]
                            )
                        else:
                            nc.vector.scalar_tensor_tensor(
                                out=ys[t][:],
                                in0=yp[:],
                                scalar=rT[h][:, t : t + 1],
                                in1=ys[t][:],
                                op0=mybir.AluOpType.mult,
                                op1=mybir.AluOpType.add,
                            )
                    return run

                def bg_y2_begin(t):
                    # head-2 Y matmul staged to SBUF; the 1/s scale + add
                    # happens in the (short) tail once rT[2] exists
                    def run():
                        yp = flex.tile([128, C], F32, tag="flex",
                                       name=f"yp2b_{t}")
                        y_mms(2, t, yp)
                        nc.vector.tensor_copy(ys2_sb[t][:], yp[:])
                    return run

                for t in range(4, NQB):
                    bg.append(bg_v_piece(t))
                for n in range(S // 512):
                    bg.append(bg_m1_piece(wq_sb, qT, True, n))
                for n in range(S // 512):
                    bg.append(bg_m1_piece(wk_sb, kT, False, n))
                bg.append(bg_dup_h2())

                for h in range(HPG):
                    for qh in range(2):
                        if h == 2 and qh == 1:
                            for t in range(NQB // 2):
                                bg.append(bg_y2_begin(t))
                        q0 = qh * 1024
                        o_ps = opp.tile([HD + 1, 1024], F32, tag="o",
                                        name=f"o_ps{h}_{qh}")
                        for kb in range(NKB):
                            kbs = slice(kb * 128, (kb + 1) * 128)
                            lt = ltp.tile([128, 1024], F32, tag="lt")
                            # two concurrent K=64 matmuls in disjoint PE
                            # row groups (rows 0-63 / 64-127)
                            nc.tensor.matmul(
                                lt[:, 0:512],
                                kT[h][0:64, kbs],
                                qT[h][0:64, q0 : q0 + 512],
                                start=True,
                                stop=True,
                            )
                            nc.tensor.matmul(
                                lt[:, 512:1024],
                                kT[h][64:128, kbs],
                                qT[h][64:128, q0 + 512 : q0 + 1024],
                                start=True,
                                stop=True,
                            )
                            elt = asb.tile([128, 1024], BF16, tag="elt")
                            nc.scalar.activation(
                                elt[:], lt[:], mybir.ActivationFunctionType.Exp
                            )
                            for nn in range(2):
                                nc.tensor.matmul(
                                    o_ps[:, nn * 512 : (nn + 1) * 512],
                                    vS[kb][:, h, :],
                                    elt[:, nn * 512 : (nn + 1) * 512],
                                    start=(kb == 0),
                                    stop=(kb == NKB - 1),
                                )
                            if bg:
                                bg.popleft()()
                        nc.vector.tensor_copy(
                            oT[h][qh][:], o_ps[0:HD, :]
                        )
                        nc.vector.tensor_copy(
                            s_row[h][:, q0 : q0 + 1024], o_ps[HD : HD + 1, :]
                        )
                    # transpose s_h to [128, 16] via DRAM bounce + 1/s
                    nc.sync.dma_start(out=s_dram[h : h + 1, :], in_=s_row[h][:])
                    nc.sync.dma_start(
                        out=sT[h][:],
                        in_=s_dram[h : h + 1, :].rearrange(
                            "a (b p) -> p (a b)", p=128
                        ),
                    )
                    nc.vector.reciprocal_approx_fast(out=rT[h][:], in_=sT[h][:])
                    if h < 2:
                        for t in range(NQB):
                            bg.append(bg_y_step(h, t))

                while bg:
                    bg.popleft()()
                # tail: finish head-2 normalization + output
                for t in range(NQB // 2):
                    nc.vector.scalar_tensor_tensor(
                        out=ys[t][:],
                        in0=ys2_sb[t][:],
                        scalar=rT[2][:, t : t + 1],
                        in1=ys[t][:],
                        op0=mybir.AluOpType.mult,
                        op1=mybir.AluOpType.add,
                    )
                    nc.sync.dma_start(
                        out=out[t * 128 : (t + 1) * 128, :], in_=ys[t][:]
                    )
                for t in range(NQB // 2, NQB):
                    yp = flex.tile([128, C], F32, tag="flex", name=f"yp2c_{t}")
                    y_mms(2, t, yp)
                    nc.vector.scalar_tensor_tensor(
                        out=ys[t][:],
                        in0=yp[:],
                        scalar=rT[2][:, t : t + 1],
                        in1=ys[t][:],
                        op0=mybir.AluOpType.mult,
                        op1=mybir.AluOpType.add,
                    )
                    nc.sync.dma_start(
                        out=out[t * 128 : (t + 1) * 128, :], in_=ys[t][:]
                    )

    nc.compile()
    return nc


_COMPILED_NC = None


def _get_nc():
    global _COMPILED_NC
    if _COMPILED_NC is None:
        _COMPILED_NC = build_program()
    return _COMPILED_NC


def make_in_maps(x, Wq, bq, Wk, bk, Wv, bv, Wo, bo):
    scale = 1.0 / np.sqrt(HD)
    bf = ml_dtypes.bfloat16
    x_bf = [np.ascontiguousarray(x[b]).astype(bf) for b in range(x.shape[0])]
    in_maps = []
    for c in range(NCORES):
        b, g = divmod(c, GROUPS)
        cols = slice(g * GF, (g + 1) * GF)
        in_maps.append(
            {
                "x": x_bf[b],
                "wq": np.ascontiguousarray(Wq[:, cols] * scale).astype(bf),
                "wk": np.ascontiguousarray(Wk[:, cols]).astype(bf),
                "wv": np.ascontiguousarray(Wv[:, cols]).astype(bf),
                "wo": np.ascontiguousarray(Wo[cols, :]).astype(bf),
                "bq": np.ascontiguousarray(bq[cols] * scale).reshape(1, GF).astype(bf),
            }
        )
    return in_maps


def gather_output(results, x, Wv, bv, Wo, bo):
    B = x.shape[0]
    out = np.zeros((B, S, C), dtype=np.float32)
    for c in range(NCORES):
        b, _ = divmod(c, GROUPS)
        out[b] += results[c]["out"]
    # exact bias folds: bk cancels in softmax; v-bias -> bv @ Wo; + bo
    out += (np.asarray(bv, np.float32) @ np.asarray(Wo, np.float32)
            + np.asarray(bo, np.float32))
    return out


def kernel(x, Wq, bq, Wk, bk, Wv, bv, Wo, bo):
    x = np.asarray(x)
    nc = _get_nc()
    in_maps = make_in_maps(x, Wq, bq, Wk, bk, Wv, bv, Wo, bo)
    res = run_bass_kernel_spmd(nc, in_maps, core_ids=list(range(NCORES)))
    return gather_output(res.results, x, Wv, bv, Wo, bo)



# revision 7
# speedup vs baseline: 1.3078x; 1.1446x over previous
"""CLIP attention (B=2, S=2048, H=768, 12 heads) on 8 trn2 NeuronCores.

Sharding: data-parallel over batch (2) x tensor-parallel over head groups
(4 groups of 3 heads).  Each core computes, for its (batch, head-group):
    q = x @ Wq_g * 1/sqrt(64) (+ bq_g scaled)      [2048, 192]
    k = x @ Wk_g                                    [2048, 192]
    v = x @ Wv_g                                    [2048, 192]
    per head: P' = exp(q k^T)   (no max subtraction; logits are O(1))
              O'^T, s via ones-augmented V:  o_ps = [V_h | 1]^T @ P'^T
    y_partial = sum_h (O'_h / s_h) @ Wo_h           [2048, 768]
      (1/s_h applied on the per-head Y psum, where the query index is the
       partition dim, via tensor_scalar with a per-partition scalar)
Host sums the 4 head-group partials per batch and adds the exactly-folded
bias terms (bk drops out of softmax; bv/bo fold to a constant row).

Matmul convention: nc.tensor.matmul(out, lhsT, rhs) => out = lhsT.T @ rhs,
contraction over the partition dim of both operands.

Scheduling/layout tricks:
  - x^T via DMA xbar transpose (bf16), split across both HWDGE queues.
  - q^T/k^T stored twice (partitions 0-63 / 64-127) so two K=64 logits
    matmuls run concurrently in disjoint PE row groups.
  - attention runs in (head, query-half) passes so its PSUM footprint is
    6 banks (2x lt + o_ps), leaving a 2-bank "flex" slot open; the head-2
    q/k projections and the per-head Y stages are drip-fed into the
    ACT-bound attention loop's PE slack through that slot.
  - s (denominators) transposed per head to [128, 16] via a DRAM bounce,
    then reciprocal_approx_fast -> per-partition scalars for Y.
"""

import sys

if "/opt/trn_rl_repo" not in sys.path:
    sys.path.insert(0, "/opt/trn_rl_repo")

from collections import deque

import numpy as np
import ml_dtypes

import concourse.bacc as bacc
import concourse.tile as tile
from concourse import mybir
from concourse.bass_utils import run_bass_kernel_spmd

BF16 = mybir.dt.bfloat16
F32 = mybir.dt.float32

S = 2048          # sequence length
C = 768           # hidden
NH = 12           # total heads
HD = 64           # head dim
NCORES = 8
GROUPS = 4        # head groups (tensor parallel)
HPG = NH // GROUPS          # heads per group = 3
GF = HPG * HD               # group feature width = 192
NCC = C // 128              # contraction chunks = 6
NQB = S // 128              # token blocks = 16
NKB = S // 128              # key blocks = 16


def build_program():
    nc = bacc.Bacc("TRN2", target_bir_lowering=False, debug=False)

    # x arrives HOST-PRE-TRANSPOSED as [C, S] so no DMA-transpose is needed
    xt_dram = nc.dram_tensor("x", (C, S), BF16, kind="ExternalInput").ap()
    wq = nc.dram_tensor("wq", (C, GF), BF16, kind="ExternalInput").ap()
    wk = nc.dram_tensor("wk", (C, GF), BF16, kind="ExternalInput").ap()
    wv = nc.dram_tensor("wv", (C, GF), BF16, kind="ExternalInput").ap()
    wo = nc.dram_tensor("wo", (GF, C), BF16, kind="ExternalInput").ap()
    bq = nc.dram_tensor("bq", (1, GF), BF16, kind="ExternalInput").ap()
    out = nc.dram_tensor("out", (S, C), F32, kind="ExternalOutput").ap()

    with tile.TileContext(nc) as tc:
        with tc.tile_pool(name="consts", bufs=1) as consts, \
             tc.tile_pool(name="dram", bufs=1, space="DRAM") as dram_pool:
            # Few LARGE prologue DMAs (many small ones serialize on the
            # recycled DMA-completion semaphores), split across both HWDGE
            # queues in consumption order.
            wq_sb = consts.tile([128, NCC, GF], BF16)
            wk_sb = consts.tile([128, NCC, GF], BF16)
            wv_sb = consts.tile([128, NCC, GF], BF16)
            wo_sb = consts.tile([HD, HPG, C], BF16)
            wo_t = [wo_sb[:, h, :] for h in range(HPG)]
            bq_sb = consts.tile([1, GF], BF16)
            xT_sb = consts.tile([128, NCC, S], BF16)
            xT = [xT_sb[:, c, :] for c in range(NCC)]
            # sync queue: x^T even chunks, wv, wo
            # scalar queue: wq, bq, x^T odd chunks, wk
            nc.sync.dma_start(out=xT_sb[:, 0, :], in_=xt_dram[0:128, :])
            nc.scalar.dma_start(
                out=wq_sb[:], in_=wq.rearrange("(c p) f -> p c f", p=128)
            )
            nc.scalar.dma_start(out=bq_sb[:], in_=bq[:])
            nc.sync.dma_start(out=xT_sb[:, 2, :], in_=xt_dram[256:384, :])
            nc.scalar.dma_start(out=xT_sb[:, 1, :], in_=xt_dram[128:256, :])
            nc.sync.dma_start(out=xT_sb[:, 4, :], in_=xt_dram[512:640, :])
            nc.scalar.dma_start(out=xT_sb[:, 3, :], in_=xt_dram[384:512, :])
            nc.sync.dma_start(
                out=wv_sb[:], in_=wv.rearrange("(c p) f -> p c f", p=128)
            )
            nc.scalar.dma_start(out=xT_sb[:, 5, :], in_=xt_dram[640:768, :])
            nc.scalar.dma_start(
                out=wk_sb[:], in_=wk.rearrange("(c p) f -> p c f", p=128)
            )
            nc.sync.dma_start(
                out=wo_sb[:], in_=wo.rearrange("(h p) f -> p h f", p=HD)
            )

            ones_bf = consts.tile([1, 512], BF16)
            nc.vector.memset(ones_bf[:], 1.0)

            qT = [consts.tile([128, S], BF16, name=f"qT{h}") for h in range(HPG)]
            kT = [consts.tile([128, S], BF16, name=f"kT{h}") for h in range(HPG)]
            vS = [consts.tile([128, HPG, HD + 1], BF16, name=f"vS{t}")
                  for t in range(NKB)]
            # oT per head, split in query halves so drip-fed consumers can
            # start after the first half is written
            oT = [[consts.tile([HD, S // 2], BF16, name=f"oT{h}_{qh}")
                   for qh in range(2)] for h in range(HPG)]
            ys2_sb = [consts.tile([128, C], F32, name=f"ys2_{t}")
                      for t in range(NQB // 2)]
            s_row = [consts.tile([1, S], F32, name=f"s_row{h}") for h in range(HPG)]
            s_dram = dram_pool.tile([HPG, S], F32)
            sT = [consts.tile([128, NQB], F32, name=f"sT{h}") for h in range(HPG)]
            rT = [consts.tile([128, NQB], F32, name=f"rT{h}") for h in range(HPG)]
            ys = [consts.tile([128, C], F32, name=f"ys{t}") for t in range(NQB)]

            def qk_proj_piece(w_sb, dst_list, with_bias, pool, tag, m0, mw, n,
                              heads):
                """One [mw, 512] projection piece -> per-head q^T/k^T rows."""
                ps = pool.tile([mw, 512], F32, tag=tag, name=f"qkp_{m0}_{n}")
                for c in range(NCC):
                    nc.tensor.matmul(
                        ps[:],
                        w_sb[:, c, m0 : m0 + mw],
                        xT[c][:, n * 512 : (n + 1) * 512],
                        start=(c == 0),
                        stop=(c == NCC - 1 and not with_bias),
                    )
                if with_bias:
                    nc.tensor.matmul(
                        ps[:], bq_sb[:, m0 : m0 + mw], ones_bf[:],
                        start=False, stop=True,
                    )
                n0 = n * 512
                for i, h in enumerate(heads):
                    nc.vector.tensor_copy(
                        dst_list[h][0:64, n0 : n0 + 512],
                        ps[i * 64 : (i + 1) * 64, :],
                    )

            def v_piece(pool, tag, t):
                vps = pool.tile([128, GF], F32, tag=tag, name=f"vp{t}")
                for c in range(NCC):
                    nc.tensor.matmul(
                        vps[:],
                        xT[c][:, t * 128 : (t + 1) * 128],
                        wv_sb[:, c, :],
                        start=(c == 0),
                        stop=(c == NCC - 1),
                    )
                nc.vector.tensor_copy(
                    vS[t][:, :, 0:HD],
                    vps[:].rearrange("p (h d) -> p h d", h=HPG),
                )
                nc.vector.memset(vS[t][:, :, HD : HD + 1], 1.0)

            # ---------------- QKV projections (heads 0,1) + V ----------------
            with tc.tile_pool(name="pp", bufs=3, space="PSUM") as pp, \
                 tc.tile_pool(name="vpp", bufs=2, space="PSUM") as vpp:
                for w_sb, dst, with_bias in ((wq_sb, qT, True), (wk_sb, kT, False)):
                    for n in range(S // 512):
                        qk_proj_piece(w_sb, dst, with_bias, pp, "pp_qk",
                                      0, 128, n, (0, 1))
                for h in (0, 1):
                    nc.sync.dma_start(out=qT[h][64:128, :], in_=qT[h][0:64, :])
                    nc.sync.dma_start(out=kT[h][64:128, :], in_=kT[h][0:64, :])
                for t in range(4):
                    v_piece(vpp, "vps", t)

            # ---------- attention + drip-fed projections/Y stages ----------
            with tc.tile_pool(name="flex", bufs=1, space="PSUM") as flex, \
                 tc.tile_pool(name="lt_ps", bufs=2, space="PSUM") as ltp, \
                 tc.tile_pool(name="o_ps", bufs=1, space="PSUM") as opp, \
                 tc.tile_pool(name="att_sb", bufs=4) as asb:
                # background tasks drip-fed into the attention loop's PE slack
                bg = deque()

                def bg_v_piece(t):
                    def run():
                        v_piece(flex, "flex", t)
                    return run

                def bg_m1_piece(w_sb, dst, with_bias, n):
                    def run():
                        qk_proj_piece(w_sb, dst, with_bias, flex, "flex",
                                      128, 64, n, (2,))
                    return run

                def bg_dup_h2():
                    def run():
                        nc.sync.dma_start(out=qT[2][64:128, :], in_=qT[2][0:64, :])
                        nc.sync.dma_start(out=kT[2][64:128, :], in_=kT[2][0:64, :])
                    return run

                def y_mms(h, t, yp):
                    tbs = slice((t % (NQB // 2)) * 128, (t % (NQB // 2) + 1) * 128)
                    for n0, nw in ((0, 512), (512, 256)):
                        nc.tensor.matmul(
                            yp[:, n0 : n0 + nw],
                            oT[h][t // (NQB // 2)][:, tbs],
                            wo_t[h][:, n0 : n0 + nw],
                            start=True,
                            stop=True,
                        )

                def bg_y_step(h, t):
                    def run():
                        yp = flex.tile([128, C], F32, tag="flex",
                                       name=f"yp{h}_{t}")
                        y_mms(h, t, yp)
                        if h == 0:
                            nc.vector.tensor_scalar_mul(
                                ys[t][:], yp[:], rT[0][:, t : t + 1]
                            )
                        else:
                            nc.vector.scalar_tensor_tensor(
                                out=ys[t][:],
                                in0=yp[:],
                                scalar=rT[h][:, t : t + 1],
                                in1=ys[t][:],
                                op0=mybir.AluOpType.mult,
                                op1=mybir.AluOpType.add,
                            )
                    return run

                def bg_y2_begin(t):
                    # head-2 Y matmul staged to SBUF; the 1/s scale + add
                    # happens in the (short) tail once rT[2] exists
                    def run():
                        yp = flex.tile([128, C], F32, tag="flex",
                                       name=f"yp2b_{t}")
                        y_mms(2, t, yp)
                        nc.vector.tensor_copy(ys2_sb[t][:], yp[:])
                    return run

                for t in range(4, NQB):
                    bg.append(bg_v_piece(t))
                for n in range(S // 512):
                    bg.append(bg_m1_piece(wq_sb, qT, True, n))
                for n in range(S // 512):
                    bg.append(bg_m1_piece(wk_sb, kT, False, n))
                bg.append(bg_dup_h2())

                for h in range(HPG):
                    for qh in range(2):
                        if h == 2 and qh == 1:
                            for t in range(NQB // 2):
                                bg.append(bg_y2_begin(t))
                        q0 = qh * 1024
                        o_ps = opp.tile([HD + 1, 1024], F32, tag="o",
                                        name=f"o_ps{h}_{qh}")
                        for kb in range(NKB):
                            kbs = slice(kb * 128, (kb + 1) * 128)
                            lt = ltp.tile([128, 1024], F32, tag="lt")
                            # two concurrent K=64 matmuls in disjoint PE
                            # row groups (rows 0-63 / 64-127)
                            nc.tensor.matmul(
                                lt[:, 0:512],
                                kT[h][0:64, kbs],
                                qT[h][0:64, q0 : q0 + 512],
                                start=True,
                                stop=True,
                            )
                            nc.tensor.matmul(
                                lt[:, 512:1024],
                                kT[h][64:128, kbs],
                                qT[h][64:128, q0 + 512 : q0 + 1024],
                                start=True,
                                stop=True,
                            )
                            elt = asb.tile([128, 1024], BF16, tag="elt")
                            nc.scalar.activation(
                                elt[:], lt[:], mybir.ActivationFunctionType.Exp
                            )
                            for nn in range(2):
                                nc.tensor.matmul(
                                    o_ps[:, nn * 512 : (nn + 1) * 512],
                                    vS[kb][:, h, :],
                                    elt[:, nn * 512 : (nn + 1) * 512],
                                    start=(kb == 0),
                                    stop=(kb == NKB - 1),
                                )
                            if bg:
                                bg.popleft()()
                        nc.vector.tensor_copy(
                            oT[h][qh][:], o_ps[0:HD, :]
                        )
                        nc.vector.tensor_copy(
                            s_row[h][:, q0 : q0 + 1024], o_ps[HD : HD + 1, :]
                        )
                    # transpose s_h to [128, 16] via DRAM bounce + 1/s
                    nc.sync.dma_start(out=s_dram[h : h + 1, :], in_=s_row[h][:])
                    nc.sync.dma_start(
                        out=sT[h][:],
                        in_=s_dram[h : h + 1, :].rearrange(
                            "a (b p) -> p (a b)", p=128
                        ),
                    )
                    nc.vector.reciprocal_approx_fast(out=rT[h][:], in_=sT[h][:])
                    if h < 2:
                        for t in range(NQB):
                            bg.append(bg_y_step(h, t))

                while bg:
                    bg.popleft()()
                # tail: finish head-2 normalization + output
                for t in range(NQB // 2):
                    nc.vector.scalar_tensor_tensor(
                        out=ys[t][:],
                        in0=ys2_sb[t][:],
                        scalar=rT[2][:, t : t + 1],
                        in1=ys[t][:],
                        op0=mybir.AluOpType.mult,
                        op1=mybir.AluOpType.add,
                    )
                    nc.sync.dma_start(
                        out=out[t * 128 : (t + 1) * 128, :], in_=ys[t][:]
                    )
                for t in range(NQB // 2, NQB):
                    yp = flex.tile([128, C], F32, tag="flex", name=f"yp2c_{t}")
                    y_mms(2, t, yp)
                    nc.vector.scalar_tensor_tensor(
                        out=ys[t][:],
                        in0=yp[:],
                        scalar=rT[2][:, t : t + 1],
                        in1=ys[t][:],
                        op0=mybir.AluOpType.mult,
                        op1=mybir.AluOpType.add,
                    )
                    nc.sync.dma_start(
                        out=out[t * 128 : (t + 1) * 128, :], in_=ys[t][:]
                    )

    nc.compile()
    return nc


_COMPILED_NC = None


def _get_nc():
    global _COMPILED_NC
    if _COMPILED_NC is None:
        _COMPILED_NC = build_program()
    return _COMPILED_NC


def make_in_maps(x, Wq, bq, Wk, bk, Wv, bv, Wo, bo):
    scale = 1.0 / np.sqrt(HD)
    bf = ml_dtypes.bfloat16
    # host-side pre-transpose: kernel takes x^T [C, S] so the device needs
    # no DMA-transposes (the XBAR + semaphore ping-pong made them slow)
    x_bf = [np.ascontiguousarray(x[b].T).astype(bf) for b in range(x.shape[0])]
    in_maps = []
    for c in range(NCORES):
        b, g = divmod(c, GROUPS)
        cols = slice(g * GF, (g + 1) * GF)
        in_maps.append(
            {
                "x": x_bf[b],
                "wq": np.ascontiguousarray(Wq[:, cols] * scale).astype(bf),
                "wk": np.ascontiguousarray(Wk[:, cols]).astype(bf),
                "wv": np.ascontiguousarray(Wv[:, cols]).astype(bf),
                "wo": np.ascontiguousarray(Wo[cols, :]).astype(bf),
                "bq": np.ascontiguousarray(bq[cols] * scale).reshape(1, GF).astype(bf),
            }
        )
    return in_maps


def gather_output(results, x, Wv, bv, Wo, bo):
    B = x.shape[0]
    out = np.zeros((B, S, C), dtype=np.float32)
    for c in range(NCORES):
        b, _ = divmod(c, GROUPS)
        out[b] += results[c]["out"]
    # exact bias folds: bk cancels in softmax; v-bias -> bv @ Wo; + bo
    out += (np.asarray(bv, np.float32) @ np.asarray(Wo, np.float32)
            + np.asarray(bo, np.float32))
    return out


def kernel(x, Wq, bq, Wk, bk, Wv, bv, Wo, bo):
    x = np.asarray(x)
    nc = _get_nc()
    in_maps = make_in_maps(x, Wq, bq, Wk, bk, Wv, bv, Wo, bo)
    res = run_bass_kernel_spmd(nc, in_maps, core_ids=list(range(NCORES)))
    return gather_output(res.results, x, Wv, bv, Wo, bo)

